# revision 14
# baseline (speedup 1.0000x reference)
"""Trainium2 Bass kernel for BackgroundSubtractorModule.

Reference computation (per 15-frame window, gray video):
  y      = 0.299 R + 0.587 G + 0.114 B            (per pixel, x scale)
  m      = mean_f y ; var = sum_f (y-m)^2 / 14
  sigma  = sqrt(var) + 1e-5
  bg     = |y - m| / sigma
  minv/maxv = min/max over pixels of bg (per frame)
  out    = (bg - minv) / (maxv - minv)  if rng > 1e-6 else bg

Sharding: 30 independent windows across 8 cores; every core runs an
identical 4-window program (cores 6,7 process one duplicated pad window
whose output is dropped).

Implementation notes (HW-measured rates drove the design):
  * Scaled luma: y' = (w0/w1) R + G + (w2/w1) B; the w1 factor is folded
    into the sigma scale and inv_sigma, so G needs no scaled copy.
    Tree shape: yf = (R-copy), p2 = (G + t2) where t2 = scaled B-copy,
    yf += p2 -- short dependency chains.
  * Frame sum accumulates on the otherwise-idle PE as identity-matmul
    PSUM accumulation (bit-exact f32, bank-aligned 512/512/128 slices);
    sum-of-squares accumulates on DVE/GPSIMD.
  * Engines run their instruction streams in order, so the program is
    software-pipelined by construction: P1 of window w+1 is emitted
    interleaved with P3/P5 of window w.
  * abs and the per-frame min/max reduces run chunked over 3-frame
    groups (FD 3456); normalize is one ACT Identity(bg*inv_rng + c) per
    frame in place; 3-frame-group stores go out on the scalar HWDGE
    queue so they interleave with the sync-queue loads.
  * Cross-partition min/max via GPSIMD partition_all_reduce(max) on
    [maxv | -minv]; inv_sigma via reciprocal_approx_accurate (2 ULP).
"""

import numpy as np
from contextlib import ExitStack

import concourse.bass as bass
import concourse.bacc as bacc
import concourse.tile as tile
from concourse import mybir, bass_isa
from concourse.bass_utils import run_bass_kernel_spmd

F32 = mybir.dt.float32
OP = mybir.AluOpType
AF = mybir.ActivationFunctionType

T, H, W = 450, 384, 384
PIX = H * W                    # 147456
WIN = 15
NCORES = 8
NWIN_CORE = 4                  # ceil(30/8) -> uniform SPMD program
FPC = NWIN_CORE * WIN          # 60 frames per core
P = 128
COLS = PIX // P                # 1152
EPS = 1e-5
THRESH = 1e-6
BANKS = ((0, 512), (512, 1024), (1024, 1152))   # PSUM bank-aligned slices

# engine-balance knobs
N_SSQ_DVE = 5          # frames whose ssq-accumulate runs on DVE (rest GPSIMD)
N_SUB_DVE = 15         # frames whose d=y-m runs on DVE (rest GPSIMD)
N_MULT_DVE = 15        # frames whose bg multiply runs on DVE (rest GPSIMD)

_BUILD_CACHE = {}


def _build(scale: float):
    w0, w1, w2 = 0.299 * scale, 0.587 * scale, 0.114 * scale
    a_r, a_b = w0 / w1, w2 / w1
    nc = bacc.Bacc("TRN2", target_bir_lowering=False, debug=False)
    vin = nc.dram_tensor("video", [FPC, PIX * 3], F32, kind="ExternalInput").ap()
    idd = nc.dram_tensor("ident", [P, P], F32, kind="ExternalInput").ap()
    vout = nc.dram_tensor("out", [FPC, PIX], F32, kind="ExternalOutput").ap()

    with tile.TileContext(nc) as tc, ExitStack() as ctx:
        p_const = ctx.enter_context(tc.tile_pool(name="const", bufs=1))
        p_y = ctx.enter_context(tc.tile_pool(name="y", bufs=2))
        p_rgb = ctx.enter_context(tc.tile_pool(name="rgb", bufs=2))
        p_stat = ctx.enter_context(tc.tile_pool(name="stat", bufs=2))
        p_tmp = ctx.enter_context(tc.tile_pool(name="tmp", bufs=5))
        p_mm = ctx.enter_context(tc.tile_pool(name="mm", bufs=2))
        p_ps = ctx.enter_context(tc.tile_pool(name="psum", bufs=1, space="PSUM"))

        ident = p_const.tile([P, P], F32)
        nc.sync.dma_start(ident[:], idd[:])

        # per-window state (created lazily per window)
        st8 = {}

        def mk_state(w):
            st8[w] = dict(
                yt=p_y.tile([P, WIN * COLS], F32, tag="y", name=f"yt{w}"),
                acc_s=p_ps.tile([P, COLS], F32, tag="acc_s", name=f"accs{w}"),
                mt=p_stat.tile([P, COLS], F32, tag="m", name=f"mt{w}"),
                st=p_stat.tile([P, COLS], F32, tag="s", name=f"st{w}"),
                mmt=p_mm.tile([P, 96], F32, tag="mm", name=f"mmt{w}"),
            )
            nc.gpsimd.memset(st8[w]["mmt"][:], 0.0)

        def yslice(w, f):
            yt = st8[w]["yt"]
            return yt[:, f * COLS:(f + 1) * COLS]

        def p1_frame(w, f):
            S = st8[w]
            g = w * WIN + f
            rgbt = p_rgb.tile([P, COLS * 3], F32, tag="rgb")
            nc.sync.dma_start(rgbt[:], vin[g].rearrange("(r j) -> r j", r=P))
            rgb3 = rgbt[:].rearrange("p (j c) -> p j c", c=3)
            yf = yslice(w, f)
            t2 = p_tmp.tile([P, COLS], F32, tag="tmp")
            nc.scalar.activation(yf, rgb3[:, :, 0], AF.Copy, bias=0.0, scale=a_r)
            nc.scalar.activation(t2[:], rgb3[:, :, 2], AF.Copy, bias=0.0, scale=a_b)
            nc.gpsimd.tensor_tensor(t2[:], t2[:], rgb3[:, :, 1], OP.add)   # G + bB
            nc.gpsimd.tensor_tensor(yf, yf, t2[:], OP.add)
            sq = p_tmp.tile([P, COLS], F32, tag="tmp")
            nc.scalar.activation(sq[:], yf, AF.Square)
            if f == 0:
                nc.vector.tensor_copy(S["st"][:], sq[:])
            else:
                eng = nc.vector if f < N_SSQ_DVE else nc.gpsimd
                eng.tensor_tensor(S["st"][:], S["st"][:], sq[:], OP.add)
            for lo, hi in BANKS:
                nc.tensor.matmul(S["acc_s"][:, lo:hi], ident[:], yf[:, lo:hi],
                                 start=(f == 0), stop=(f == WIN - 1))

        def p2(w):
            S = st8[w]
            mt, st, = S["mt"], S["st"]
            nc.vector.tensor_scalar(mt[:], S["acc_s"][:], 1.0 / WIN, None, OP.mult)
            msq = p_tmp.tile([P, COLS], F32, tag="tmp")
            nc.scalar.activation(msq[:], mt[:], AF.Square, scale=float(np.sqrt(15.0)))
            nc.vector.tensor_tensor(st[:], st[:], msq[:], OP.subtract)
            nc.scalar.activation(st[:], st[:], AF.Sqrt, scale=w1 * w1 / (WIN - 1))
            # recip input: (sigma + eps)/w1  ->  recip = w1/(sigma+eps)
            nc.vector.tensor_scalar(st[:], st[:], EPS, 1.0 / w1, OP.add, OP.mult)
            scr = p_tmp.tile([P, COLS], F32, tag="tmp")
            nc.vector.reciprocal_approx_accurate(st[:], st[:], scr[:])

        def p3_group(w, grp):
            S = st8[w]
            f0 = grp * 3
            for f in range(f0, f0 + 3):
                eng = nc.vector if f < N_SUB_DVE else nc.gpsimd
                eng.tensor_tensor(yslice(w, f), yslice(w, f), S["mt"][:], OP.subtract)
            ych = S["yt"][:, f0 * COLS:(f0 + 3) * COLS]
            nc.scalar.activation(ych, ych, AF.Abs)
            for f in range(f0, f0 + 3):
                eng = nc.vector if f < N_MULT_DVE else nc.gpsimd
                eng.tensor_tensor(yslice(w, f), yslice(w, f), S["st"][:], OP.mult)
            ych3 = ych.rearrange("p (f j) -> p f j", f=3)
            mmt = S["mmt"]
            nc.vector.tensor_reduce(
                mmt[:, f0:f0 + 3], ych3, axis=mybir.AxisListType.X, op=OP.max)
            nc.vector.tensor_reduce(
                mmt[:, 16 + f0:19 + f0], ych3, axis=mybir.AxisListType.X, op=OP.min)

        def p4(w):
            mmt = st8[w]["mmt"]
            nc.vector.tensor_scalar(mmt[:, 16:32], mmt[:, 16:32], -1.0, None, OP.mult)
            nc.gpsimd.partition_all_reduce(
                mmt[:, 32:64], mmt[:, 0:32], 128, bass_isa.ReduceOp.max
            )
            mx, nmn = mmt[:, 32:48], mmt[:, 48:64]
            rng, msk = mmt[:, 64:80], mmt[:, 80:96]
            nc.vector.tensor_tensor(rng, mx, nmn, OP.add)            # maxv - minv
            nc.vector.tensor_scalar(msk, rng, THRESH, None, OP.is_gt)
            nc.vector.tensor_tensor(rng, rng, msk, OP.mult)
            nc.vector.tensor_scalar(rng, rng, 1.0, None, OP.add)
            nc.vector.tensor_tensor(rng, rng, msk, OP.subtract)      # rng_safe
            nc.vector.reciprocal(rng, rng)                           # inv_rng
            c1 = mmt[:, 0:16]
            nc.vector.tensor_tensor(c1, nmn, msk, OP.mult)           # -minv_eff
            nc.vector.tensor_tensor(c1, c1, rng, OP.mult)            # *inv_rng

        def p5_group(w, grp):
            S = st8[w]
            mmt = S["mmt"]
            rng, c1 = mmt[:, 64:80], mmt[:, 0:16]
            f0 = grp * 3
            for f in range(f0, f0 + 3):
                nc.scalar.activation(
                    yslice(w, f), yslice(w, f), AF.Identity,
                    bias=c1[:, f:f + 1], scale=rng[:, f:f + 1]
                )
            g0 = w * WIN + f0
            nc.scalar.dma_start(
                vout[g0:g0 + 3].rearrange("f (r j) -> r f j", r=P),
                S["yt"][:, f0 * COLS:(f0 + 3) * COLS].rearrange(
                    "p (f j) -> p f j", f=3),
            )

        # ---- software-pipelined emission ----
        mk_state(0)
        for f in range(WIN):
            p1_frame(0, f)
        for w in range(NWIN_CORE):
            nxt = w + 1 if w + 1 < NWIN_CORE else None
            if nxt is not None:
                mk_state(nxt)
            p2(w)
            for grp in range(5):
                p3_group(w, grp)
                if nxt is not None:
                    p1_frame(nxt, grp * 2)
                    p1_frame(nxt, grp * 2 + 1)
            p4(w)
            for grp in range(5):
                p5_group(w, grp)
                if nxt is not None and 10 + grp < WIN:
                    p1_frame(nxt, 10 + grp)
            del st8[w]

    nc.compile()
    return nc


def _get_nc(scale: float):
    key = round(float(scale), 9)
    if key not in _BUILD_CACHE:
        _BUILD_CACHE[key] = _build(key)
    return _BUILD_CACHE[key]


def kernel(video: np.ndarray) -> np.ndarray:
    video = np.ascontiguousarray(np.asarray(video, dtype=np.float32))
    assert video.shape == (T, H, W, 3), video.shape
    scale = 1.0 / 255.0 if float(video.max()) > 1.0 else 1.0
    nc = _get_nc(scale)

    v = video.reshape(T, PIX * 3)
    shards = []
    for c in range(6):
        shards.append(v[c * FPC:(c + 1) * FPC])
    # cores 6,7: 3 real windows + last window repeated as pad
    shards.append(np.concatenate([v[360:405], v[390:405]], axis=0))
    shards.append(np.concatenate([v[405:450], v[435:450]], axis=0))

    ident = np.eye(P, dtype=np.float32)
    res = run_bass_kernel_spmd(
        nc, [{"video": s, "ident": ident} for s in shards], list(range(NCORES))
    )
    outs = [res.results[c]["out"] for c in range(NCORES)]
    full = np.concatenate(
        [o[:FPC] for o in outs[:6]] + [outs[6][:45], outs[7][:45]], axis=0
    )
    return full.reshape(T, 1, H, W)
